# revision 18
# baseline (speedup 1.0000x reference)
"""Trainium2 Bass kernel for nn_CLAM_SB (gated-attention MIL, topk instance mining).

Strategy (8 NeuronCores, instance dim N=100000 sharded 12500 rows/core):

  Device (per core, fp16 matmuls / fp32 accumulation):
    h1^T = relu(W1^T @ h^T + b1)          [512, 12500]  (D1 on partitions)
    a^T  = tanh(Wa^T @ h1^T + ba)         [256, 12500]
    g^T  = sigmoid(Wb^T @ h1^T + bb)      [256, 12500]
    A    = Wc^T @ (a*g)^T + bc            [1, 12500]   -> output A_raw shard
    E    = exp(A)                          broadcast to 128 partitions via PE
    M_raw[d] = sum_r E_r * h1^T[d, r]      (fused DVE tensor_tensor_reduce)

  Host (numpy fp32):
    - shard + cast h to fp16, pre-transpose per core (so no on-device transpose)
    - A_raw = concat of shards; Z = sum exp(A_raw)
    - pooled M = (sum_c M_raw_c)/Z; logits/Y_prob/Y_hat from M @ Wbag
    - top-k: candidates = global top-64/bottom-64 of device A_raw, then the
      candidate rows are recomputed exactly in fp32 (tiny: 128 rows) and the
      final top-8/bottom-8 + inst_logits come from that exact recompute.
      Device noise (~1e-4) is far below the candidate margin, and the final
      selection/ordering matches the fp32 reference exactly.
"""

import numpy as np
from contextlib import ExitStack

# Problem constants (hardcoded per harness contract).
N, L, D1, D2, TOPK, NCLS = 100000, 1024, 512, 256, 8, 2
NCORES = 8
R = N // NCORES           # 12500 rows per core
BLK = 512                 # rows per block (matmul moving dim / PSUM bank)
NKC = L // 128            # 8 contraction chunks for h @ W1
ND1 = D1 // 128           # 4 D1 chunks
ND2 = D2 // 128           # 2 D2 chunks
NCAND = 64                # top/bottom candidates refined on host

_prog_cache = {}


# ---------------------------------------------------------------------------
# Wait-splitting post-pass: the walrus build in this container rejects
# instructions whose sync_info carries more than one wait ("Too many sync
# wait commands"). Tile freely emits multi-waits; rewrite every instruction
# with k>1 waits into k-1 preceding single-wait NOPs on the same engine.
# Per-engine program order makes this semantically identical.
# ---------------------------------------------------------------------------
def _split_multi_waits(nc):
    import bass_rust
    import concourse.mybir as mybir

    engine_attr = {
        mybir.EngineType.PE: "tensor",
        mybir.EngineType.DVE: "vector",
        mybir.EngineType.Activation: "scalar",
        mybir.EngineType.Pool: "gpsimd",
        mybir.EngineType.SP: "sync",
    }

    def make_wait_nop(engine, wait):
        eng = getattr(nc, engine_attr[engine])
        inst = eng.nop(nofuse=True).ins
        for fn in nc.m.functions:
            for bb in fn.blocks:
                if inst in bb.instructions:
                    bb.instructions.remove(inst)
        inst.sync_info = bass_rust.SyncInfo(on_wait=[wait], on_update=[])
        return inst

    for fn in nc.m.functions:
        for bb in fn.blocks:
            new_insts = []
            for inst in bb.instructions:
                si = inst.sync_info
                if si is not None and si.on_wait and len(si.on_wait) > 1:
                    waits = list(si.on_wait)
                    for w in waits[:-1]:
                        new_insts.append(make_wait_nop(inst.engine, w))
                    inst.sync_info = bass_rust.SyncInfo(
                        on_wait=[waits[-1]], on_update=list(si.on_update or [])
                    )
                new_insts.append(inst)
            bb.instructions[:] = new_insts


def _build_program():
    """Build the per-core SPMD Bass program (same NEFF for all 8 cores)."""
    import concourse.bass as bass
    import concourse.tile as tile
    import concourse.mybir as mybir

    f16, f32 = mybir.dt.float16, mybir.dt.float32
    mult, add = mybir.AluOpType.mult, mybir.AluOpType.add
    AF = mybir.ActivationFunctionType

    nc = bass.Bass("TRN2", debug=False)

    hT = nc.dram_tensor("hT", [L, R], f16, kind="ExternalInput").ap()
    W1f = nc.dram_tensor("W1f", [L, D1], f16, kind="ExternalInput").ap()
    Waf = nc.dram_tensor("Waf", [D1, D2], f16, kind="ExternalInput").ap()
    Wbf = nc.dram_tensor("Wbf", [D1, D2], f16, kind="ExternalInput").ap()
    Wcf = nc.dram_tensor("Wcf", [D2, 1], f16, kind="ExternalInput").ap()
    b1f = nc.dram_tensor("b1f", [128, ND1], f32, kind="ExternalInput").ap()
    baf = nc.dram_tensor("baf", [128, ND2], f32, kind="ExternalInput").ap()
    bbf = nc.dram_tensor("bbf", [128, ND2], f32, kind="ExternalInput").ap()
    bcf = nc.dram_tensor("bcf", [1, 1], f32, kind="ExternalInput").ap()

    A_out = nc.dram_tensor("A_out", [1, R], f32, kind="ExternalOutput").ap()
    M_out = nc.dram_tensor("M_out", [128, ND1], f32, kind="ExternalOutput").ap()

    # Block schedule: four 128-row mini blocks first (PE starts after only
    # ~1.25MB of DMA instead of ~3MB), then 512-row blocks. h^T DMAs are
    # grouped (mini blocks singly, full blocks in pairs -> 2KB partition
    # lines) and prefetched two groups ahead.
    blocks = [(i * 128, 128) for i in range(4)]          # (r0, B)
    r = 512
    while r < R:
        B = min(BLK, R - r)
        blocks.append((r, B))
        r += B
    nblk = len(blocks)
    groups = [[b] for b in range(4)]                     # group -> block idxs
    b = 4
    while b < nblk:
        groups.append([b] if b + 1 >= nblk else [b, b + 1])
        b += 2
    ngrp = len(groups)
    gidx = {}
    goff = {}
    for g, bs in enumerate(groups):
        for j, bi in enumerate(bs):
            gidx[bi] = g
            goff[bi] = blocks[bi][0] - blocks[bs[0]][0]

    with tile.TileContext(nc) as tc, ExitStack() as ctx:
        wpool = ctx.enter_context(tc.tile_pool(name="weights", bufs=1))
        hpool = ctx.enter_context(tc.tile_pool(name="ht", bufs=1))
        cpool = ctx.enter_context(tc.tile_pool(name="compute", bufs=1))
        ppool = ctx.enter_context(tc.tile_pool(name="psum", bufs=1, space="PSUM"))

        # --- persistent weights/biases in SBUF ---
        # W1 tiles are interleaved with the first h^T block so the PE's first
        # matmul dependencies finish loading as early as possible; everything
        # only needed from pipeline stage 2 onward loads afterwards.
        ht_groups = {}   # g -> list of NKC tiles [128, 2*BLK]

        def load_group(g, interleave_W1=None):
            bs = groups[g]
            r0 = blocks[bs[0]][0]
            w = blocks[bs[-1]][0] + blocks[bs[-1]][1] - r0
            tiles = []
            for kc in range(NKC):
                if interleave_W1 is not None:
                    nc.sync.dma_start(
                        interleave_W1[:, kc * D1:(kc + 1) * D1],
                        W1f[kc * 128:(kc + 1) * 128, :])
                t = hpool.tile([128, 2 * BLK], f16, tag=f"ht{kc}", bufs=4)
                nc.sync.dma_start(
                    t[:, :w], hT[kc * 128:(kc + 1) * 128, r0:r0 + w])
                tiles.append(t)
            ht_groups[g] = tiles

        # W1 tiles interleaved with the first h^T group so the PE's first
        # matmul dependencies finish loading as early as possible.
        W1sb = wpool.tile([128, NKC * D1], f16)          # [k, kc*512 + j]
        load_group(0, interleave_W1=W1sb)
        for g in range(1, min(4, ngrp)):
            load_group(g)
        Wasb = wpool.tile([128, ND1 * D2], f16)          # [k, d1c*256 + j]
        Wbsb = wpool.tile([128, ND1 * D2], f16)
        for d1c in range(ND1):
            nc.sync.dma_start(Wasb[:, d1c * D2:(d1c + 1) * D2],
                              Waf[d1c * 128:(d1c + 1) * 128, :])
            nc.sync.dma_start(Wbsb[:, d1c * D2:(d1c + 1) * D2],
                              Wbf[d1c * 128:(d1c + 1) * 128, :])
        Wcsb = wpool.tile([128, ND2], f16)               # [k, d2c]
        for d2c in range(ND2):
            nc.sync.dma_start(Wcsb[:, d2c:d2c + 1],
                              Wcf[d2c * 128:(d2c + 1) * 128, :])
        b1sb = wpool.tile([128, ND1], f32)
        nc.sync.dma_start(b1sb[:], b1f[:, :])
        basb = wpool.tile([128, ND2], f32)
        nc.sync.dma_start(basb[:], baf[:, :])
        bbsb = wpool.tile([128, ND2], f32)
        nc.sync.dma_start(bbsb[:], bbf[:, :])
        bcsb = wpool.tile([1, 1], f32)
        nc.sync.dma_start(bcsb[:], bcf[:, :])

        # --- persistent accumulators / staged outputs ---
        A_sb = wpool.tile([1, R], f32)
        Macc = wpool.tile([128, ND1], f32)
        # per-block pooled partial sums; reduced into Macc at the end
        Msum = [wpool.tile([128, nblk], f32, name=f"Msum{d1c}")
                for d1c in range(ND1)]

        # 4-stage software pipeline, one block of skew between stages, so the
        # PE instruction stream never waits on ACT/DVE results of the same
        # block: stage1(b)=load+h1, stage2(b-1)=a/g/s, stage3(b-2)=A+exp,
        # stage4(b-3)=E broadcast + pooled partial.
        st_h1 = {}   # b -> list of 4 h1^T tiles
        st_s = {}    # b -> list of 2 s tiles
        st_E = {}    # b -> Et tile

        def blk_of(b):
            return blocks[b][1]

        for it in range(nblk + 3):
            if it < nblk:
                b, B = it, blk_of(it)
                g, off = gidx[b], goff[b]
                if b == groups[g][0] and g + 2 < ngrp and g + 2 not in ht_groups:
                    load_group(g + 2)
                ht = ht_groups[g]
                h1 = []
                for d1c in range(ND1):
                    ps = ppool.tile([128, BLK], f32, tag="ph1", bufs=3)
                    for kc in range(NKC):
                        lo = kc * D1 + d1c * 128
                        nc.tensor.matmul(ps[:, :B], W1sb[:, lo:lo + 128],
                                         ht[kc][:, off:off + B],
                                         start=(kc == 0), stop=(kc == NKC - 1))
                    t = cpool.tile([128, BLK], f16, tag=f"h1_{d1c}", bufs=5)
                    nc.scalar.activation(t[:, :B], ps[:, :B], AF.Relu,
                                         bias=b1sb[:, d1c:d1c + 1])
                    h1.append(t)
                st_h1[b] = h1

            if 0 <= it - 1 < nblk:
                b, B = it - 1, blk_of(it - 1)
                h1 = st_h1[b]
                s = []
                for d2c in range(ND2):
                    pa = ppool.tile([128, BLK], f32, tag="pag", bufs=3)
                    for d1c in range(ND1):
                        lo = d1c * D2 + d2c * 128
                        nc.tensor.matmul(pa[:, :B], Wasb[:, lo:lo + 128],
                                         h1[d1c][:, :B],
                                         start=(d1c == 0), stop=(d1c == ND1 - 1))
                    at = cpool.tile([128, BLK], f16, tag=f"a_{d2c}", bufs=2)
                    nc.scalar.activation(at[:, :B], pa[:, :B], AF.Tanh,
                                         bias=basb[:, d2c:d2c + 1])

                    pg = ppool.tile([128, BLK], f32, tag="pag", bufs=3)
                    for d1c in range(ND1):
                        lo = d1c * D2 + d2c * 128
                        nc.tensor.matmul(pg[:, :B], Wbsb[:, lo:lo + 128],
                                         h1[d1c][:, :B],
                                         start=(d1c == 0), stop=(d1c == ND1 - 1))
                    gt = cpool.tile([128, BLK], f16, tag=f"g_{d2c}", bufs=2)
                    nc.scalar.activation(gt[:, :B], pg[:, :B], AF.Sigmoid,
                                         bias=bbsb[:, d2c:d2c + 1])

                    st = cpool.tile([128, BLK], f16, tag=f"s_{d2c}", bufs=3)
                    nc.vector.tensor_mul(st[:, :B], at[:, :B], gt[:, :B])
                    s.append(st)
                st_s[b] = s

            if 0 <= it - 2 < nblk:
                b = it - 2
                r0, B = blocks[b]
                s = st_s.pop(b)
                pA = ppool.tile([1, BLK], f32, tag="pA", bufs=2)
                for d2c in range(ND2):
                    nc.tensor.matmul(pA[:1, :B], Wcsb[:, d2c:d2c + 1],
                                     s[d2c][:, :B],
                                     start=(d2c == 0), stop=(d2c == ND2 - 1))
                nc.scalar.activation(A_sb[:1, r0:r0 + B], pA[:1, :B], AF.Identity,
                                     bias=bcsb[:1, 0:1])
                Et = cpool.tile([1, BLK], f16, tag="E", bufs=3)
                nc.scalar.activation(Et[:1, :B], pA[:1, :B], AF.Exp,
                                     bias=bcsb[:1, 0:1])
                st_E[b] = Et

            if 0 <= it - 3 < nblk:
                b, B = it - 3, blk_of(it - 3)
                h1 = st_h1.pop(b)
                Et = st_E.pop(b)
                # broadcast E to all 128 partitions via SWDGE DMA (0-step
                # free dim on the source) — keeps the PE stream pure matmul
                Es = cpool.tile([128, BLK], f16, tag="Es", bufs=2)
                src = (Et[0:1, :B].rearrange("p (a f) -> p a f", a=1)
                       .broadcast_to((1, 128, B)))
                nc.sync.dma_start(Es[:, :B], src)
                for d1c in range(ND1):
                    tmp = cpool.tile([128, BLK], f32, tag="pooltmp", bufs=2)
                    nc.vector.scalar_tensor_tensor(
                        out=tmp[:, :B], in0=h1[d1c][:, :B], scalar=1.0,
                        in1=Es[:, :B], op0=mult, op1=mult,
                        accum_out=Msum[d1c][:, b:b + 1])

        for d1c in range(ND1):
            nc.vector.reduce_sum(Macc[:, d1c:d1c + 1], Msum[d1c][:, :],
                                 axis=mybir.AxisListType.X)

        nc.sync.dma_start(A_out[:, :], A_sb[:1, :])
        nc.sync.dma_start(M_out[:, :], Macc[:, :])

    _split_multi_waits(nc)
    return nc


def _run_device(h):
    """Shard/cast/transpose h, run the SPMD kernel, return (A_raw[N], M_raw[512])
    plus the per-core weight inputs captured in _run_device.weights."""
    from concourse.bass_utils import run_bass_kernel_spmd

    if "nc" not in _prog_cache:
        _prog_cache["nc"] = _build_program()
    nc = _prog_cache["nc"]

    w = _run_device.weights
    in_maps = []
    for c in range(NCORES):
        shard = h[c * R:(c + 1) * R, :]
        hT_c = np.ascontiguousarray(shard.astype(np.float16).T)
        in_maps.append({"hT": hT_c, **w})

    res = run_bass_kernel_spmd(nc, in_maps, core_ids=list(range(NCORES)))
    A_raw = np.concatenate([res.results[c]["A_out"][0] for c in range(NCORES)])
    M_raw = np.zeros(D1, np.float64)
    for c in range(NCORES):
        M_raw += res.results[c]["M_out"].T.reshape(D1)
    return A_raw, M_raw


_run_device.weights = None


def kernel(h, W1, b1, Wa, ba, Wb, bb, Wc, bc, Wbag, bbag, Winst, binst, label):
    h = np.asarray(h, dtype=np.float32)
    W1 = np.asarray(W1, dtype=np.float32)
    b1 = np.asarray(b1, dtype=np.float32)
    Wa = np.asarray(Wa, dtype=np.float32)
    ba = np.asarray(ba, dtype=np.float32)
    Wb = np.asarray(Wb, dtype=np.float32)
    bb = np.asarray(bb, dtype=np.float32)
    Wc = np.asarray(Wc, dtype=np.float32)
    bc = np.asarray(bc, dtype=np.float32)
    Wbag = np.asarray(Wbag, dtype=np.float32)
    bbag = np.asarray(bbag, dtype=np.float32)
    Winst = np.asarray(Winst, dtype=np.float32)
    binst = np.asarray(binst, dtype=np.float32)

    _run_device.weights = {
        "W1f": W1.astype(np.float16),
        "Waf": Wa.astype(np.float16),
        "Wbf": Wb.astype(np.float16),
        "Wcf": Wc.astype(np.float16),
        "b1f": np.ascontiguousarray(b1.reshape(ND1, 128).T.astype(np.float32)),
        "baf": np.ascontiguousarray(ba.reshape(ND2, 128).T.astype(np.float32)),
        "bbf": np.ascontiguousarray(bb.reshape(ND2, 128).T.astype(np.float32)),
        "bcf": bc.reshape(1, 1).astype(np.float32),
    }

    A_raw_dev, M_raw = _run_device(h)  # [N] f32 (device), [512] f64 partials

    # --- bag branch (host fp32, negligible cost) ---
    Aexp = np.exp(A_raw_dev.astype(np.float64))
    Z = Aexp.sum()
    M = (M_raw / Z).astype(np.float32)                    # [512] pooled vector
    logits = (M @ Wbag + bbag).reshape(1, NCLS).astype(np.float32)
    lmax = logits.max(axis=1, keepdims=True)
    e = np.exp(logits - lmax)
    Y_prob = (e / e.sum(axis=1, keepdims=True)).astype(np.float32)
    Y_hat = np.argmax(logits, axis=1).astype(np.int32)

    # --- instance branch: exact fp32 refinement of candidates ---
    top_cand = np.argpartition(-A_raw_dev, NCAND - 1)[:NCAND]
    bot_cand = np.argpartition(A_raw_dev, NCAND - 1)[:NCAND]
    cand = np.unique(np.concatenate([top_cand, bot_cand]))

    h_sel = h[cand]                                        # [|cand|, 1024]
    h1_sel = np.maximum(h_sel @ W1 + b1, 0.0).astype(np.float32)
    a_sel = np.tanh(h1_sel @ Wa + ba)
    g_sel = 1.0 / (1.0 + np.exp(-(h1_sel @ Wb + bb)))
    A_sel = ((a_sel * g_sel).astype(np.float32) @ Wc + bc).reshape(-1).astype(np.float32)

    # top_k on softmax scores == top_k on A (softmax monotonic); lax.top_k
    # breaks ties toward the lower index.
    order_desc = np.lexsort((cand, -A_sel))
    order_asc = np.lexsort((cand, A_sel))
    top_rows = order_desc[:TOPK]
    bot_rows = order_asc[:TOPK]
    sel_rows = np.concatenate([top_rows, bot_rows])
    all_inst = h1_sel[sel_rows]                            # [2K, 512] exact fp32
    inst_logits = (all_inst @ Winst + binst).astype(np.float32)

    A_raw = A_raw_dev.reshape(1, N).astype(np.float32)
    return (logits, Y_prob, Y_hat, A_raw, inst_logits)


# revision 20
# speedup vs baseline: 1.0716x; 1.0716x over previous
"""Trainium2 Bass kernel for nn_CLAM_SB (gated-attention MIL, topk instance mining).

Strategy (8 NeuronCores, instance dim N=100000 sharded 12500 rows/core):

  Device (per core, fp16 matmuls / fp32 accumulation):
    h1^T = relu(W1^T @ h^T + b1)          [512, 12500]  (D1 on partitions)
    a^T  = tanh(Wa^T @ h1^T + ba)         [256, 12500]
    g^T  = sigmoid(Wb^T @ h1^T + bb)      [256, 12500]
    A    = Wc^T @ (a*g)^T + bc            [1, 12500]   -> output A_raw shard
    E    = exp(A)                          broadcast to 128 partitions via PE
    M_raw[d] = sum_r E_r * h1^T[d, r]      (fused DVE tensor_tensor_reduce)

  Host (numpy fp32):
    - shard + cast h to fp16, pre-transpose per core (so no on-device transpose)
    - A_raw = concat of shards; Z = sum exp(A_raw)
    - pooled M = (sum_c M_raw_c)/Z; logits/Y_prob/Y_hat from M @ Wbag
    - top-k: candidates = global top-64/bottom-64 of device A_raw, then the
      candidate rows are recomputed exactly in fp32 (tiny: 128 rows) and the
      final top-8/bottom-8 + inst_logits come from that exact recompute.
      Device noise (~1e-4) is far below the candidate margin, and the final
      selection/ordering matches the fp32 reference exactly.
"""

import numpy as np
from contextlib import ExitStack

# Problem constants (hardcoded per harness contract).
N, L, D1, D2, TOPK, NCLS = 100000, 1024, 512, 256, 8, 2
NCORES = 8
R = N // NCORES           # 12500 rows per core
BLK = 512                 # rows per block (matmul moving dim / PSUM bank)
NKC = L // 128            # 8 contraction chunks for h @ W1
ND1 = D1 // 128           # 4 D1 chunks
ND2 = D2 // 128           # 2 D2 chunks
NCAND = 64                # top/bottom candidates refined on host

_prog_cache = {}


# ---------------------------------------------------------------------------
# Wait-splitting post-pass: the walrus build in this container rejects
# instructions whose sync_info carries more than one wait ("Too many sync
# wait commands"). Tile freely emits multi-waits; rewrite every instruction
# with k>1 waits into k-1 preceding single-wait NOPs on the same engine.
# Per-engine program order makes this semantically identical.
# ---------------------------------------------------------------------------
def _split_multi_waits(nc):
    import bass_rust
    import concourse.mybir as mybir

    engine_attr = {
        mybir.EngineType.PE: "tensor",
        mybir.EngineType.DVE: "vector",
        mybir.EngineType.Activation: "scalar",
        mybir.EngineType.Pool: "gpsimd",
        mybir.EngineType.SP: "sync",
    }

    def make_wait_nop(engine, wait):
        eng = getattr(nc, engine_attr[engine])
        inst = eng.nop(nofuse=True).ins
        for fn in nc.m.functions:
            for bb in fn.blocks:
                if inst in bb.instructions:
                    bb.instructions.remove(inst)
        inst.sync_info = bass_rust.SyncInfo(on_wait=[wait], on_update=[])
        return inst

    for fn in nc.m.functions:
        for bb in fn.blocks:
            new_insts = []
            for inst in bb.instructions:
                si = inst.sync_info
                if si is not None and si.on_wait and len(si.on_wait) > 1:
                    waits = list(si.on_wait)
                    for w in waits[:-1]:
                        new_insts.append(make_wait_nop(inst.engine, w))
                    inst.sync_info = bass_rust.SyncInfo(
                        on_wait=[waits[-1]], on_update=list(si.on_update or [])
                    )
                new_insts.append(inst)
            bb.instructions[:] = new_insts


def _build_program():
    """Build the per-core SPMD Bass program (same NEFF for all 8 cores)."""
    import concourse.bass as bass
    import concourse.tile as tile
    import concourse.mybir as mybir

    f16, f32 = mybir.dt.float16, mybir.dt.float32
    mult, add = mybir.AluOpType.mult, mybir.AluOpType.add
    AF = mybir.ActivationFunctionType

    nc = bass.Bass("TRN2", debug=False)

    hT = nc.dram_tensor("hT", [L, R], f16, kind="ExternalInput").ap()
    # weights arrive pre-arranged as their SBUF images (one contiguous,
    # descriptor-efficient DMA each): W1f[p, kc*D1+j] = W1[kc*128+p, j] etc.
    W1f = nc.dram_tensor("W1f", [128, NKC * D1], f16, kind="ExternalInput").ap()
    Waf = nc.dram_tensor("Waf", [128, ND1 * D2], f16, kind="ExternalInput").ap()
    Wbf = nc.dram_tensor("Wbf", [128, ND1 * D2], f16, kind="ExternalInput").ap()
    Wcf = nc.dram_tensor("Wcf", [128, ND2], f16, kind="ExternalInput").ap()
    b1f = nc.dram_tensor("b1f", [128, ND1], f32, kind="ExternalInput").ap()
    baf = nc.dram_tensor("baf", [128, ND2], f32, kind="ExternalInput").ap()
    bbf = nc.dram_tensor("bbf", [128, ND2], f32, kind="ExternalInput").ap()
    bcf = nc.dram_tensor("bcf", [1, 1], f32, kind="ExternalInput").ap()

    A_out = nc.dram_tensor("A_out", [1, R], f32, kind="ExternalOutput").ap()
    M_out = nc.dram_tensor("M_out", [128, ND1], f32, kind="ExternalOutput").ap()

    # Block schedule: 512-row blocks; h^T DMAs grouped in pairs of blocks
    # (2KB partition lines for descriptor efficiency), prefetched two groups
    # ahead of compute.
    blocks = []                                          # (r0, B)
    r = 0
    while r < R:
        B = min(BLK, R - r)
        blocks.append((r, B))
        r += B
    nblk = len(blocks)
    groups = []                                          # group -> block idxs
    b = 0
    while b < nblk:
        groups.append([b] if b + 1 >= nblk else [b, b + 1])
        b += 2
    ngrp = len(groups)
    gidx = {}
    goff = {}
    for g, bs in enumerate(groups):
        for j, bi in enumerate(bs):
            gidx[bi] = g
            goff[bi] = blocks[bi][0] - blocks[bs[0]][0]

    with tile.TileContext(nc) as tc, ExitStack() as ctx:
        wpool = ctx.enter_context(tc.tile_pool(name="weights", bufs=1))
        hpool = ctx.enter_context(tc.tile_pool(name="ht", bufs=1))
        cpool = ctx.enter_context(tc.tile_pool(name="compute", bufs=1))
        ppool = ctx.enter_context(tc.tile_pool(name="psum", bufs=1, space="PSUM"))

        # --- persistent weights/biases in SBUF ---
        # W1 tiles are interleaved with the first h^T block so the PE's first
        # matmul dependencies finish loading as early as possible; everything
        # only needed from pipeline stage 2 onward loads afterwards.
        ht_groups = {}   # g -> list of NKC tiles [128, 2*BLK]

        def load_group(g):
            bs = groups[g]
            r0 = blocks[bs[0]][0]
            w = blocks[bs[-1]][0] + blocks[bs[-1]][1] - r0
            tiles = []
            for kc in range(NKC):
                t = hpool.tile([128, 2 * BLK], f16, tag=f"ht{kc}", bufs=3)
                nc.sync.dma_start(
                    t[:, :w], hT[kc * 128:(kc + 1) * 128, r0:r0 + w])
                tiles.append(t)
            ht_groups[g] = tiles

        # W1 (one contiguous 8KB-per-line DMA) then the first two h^T groups.
        W1sb = wpool.tile([128, NKC * D1], f16)          # [k, kc*512 + j]
        nc.sync.dma_start(W1sb[:], W1f[:, :])
        for g in range(min(2, ngrp)):
            load_group(g)
        Wasb = wpool.tile([128, ND1 * D2], f16)          # [k, d1c*256 + j]
        nc.sync.dma_start(Wasb[:], Waf[:, :])
        Wbsb = wpool.tile([128, ND1 * D2], f16)
        nc.sync.dma_start(Wbsb[:], Wbf[:, :])
        Wcsb = wpool.tile([128, ND2], f16)               # [k, d2c]
        nc.sync.dma_start(Wcsb[:], Wcf[:, :])
        b1sb = wpool.tile([128, ND1], f32)
        nc.sync.dma_start(b1sb[:], b1f[:, :])
        basb = wpool.tile([128, ND2], f32)
        nc.sync.dma_start(basb[:], baf[:, :])
        bbsb = wpool.tile([128, ND2], f32)
        nc.sync.dma_start(bbsb[:], bbf[:, :])
        bcsb = wpool.tile([1, 1], f32)
        nc.sync.dma_start(bcsb[:], bcf[:, :])

        # --- persistent accumulators / staged outputs ---
        A_sb = wpool.tile([1, R], f32)
        Macc = wpool.tile([128, ND1], f32)
        # per-block pooled partial sums; reduced into Macc at the end
        Msum = [wpool.tile([128, nblk], f32, name=f"Msum{d1c}")
                for d1c in range(ND1)]

        # 4-stage software pipeline, one block of skew between stages, so the
        # PE instruction stream never waits on ACT/DVE results of the same
        # block: stage1(b)=load+h1, stage2(b-1)=a/g/s, stage3(b-2)=A+exp,
        # stage4(b-3)=E broadcast + pooled partial.
        st_h1 = {}   # b -> list of 4 h1^T tiles
        st_s = {}    # b -> list of 2 s tiles
        st_E = {}    # b -> Et tile

        def blk_of(b):
            return blocks[b][1]

        for it in range(nblk + 3):
            if it < nblk:
                b, B = it, blk_of(it)
                g, off = gidx[b], goff[b]
                if b == groups[g][0] and g + 2 < ngrp and g + 2 not in ht_groups:
                    load_group(g + 2)
                ht = ht_groups[g]
                h1 = []
                for d1c in range(ND1):
                    ps = ppool.tile([128, BLK], f32, tag="ph1", bufs=3)
                    for kc in range(NKC):
                        lo = kc * D1 + d1c * 128
                        nc.tensor.matmul(ps[:, :B], W1sb[:, lo:lo + 128],
                                         ht[kc][:, off:off + B],
                                         start=(kc == 0), stop=(kc == NKC - 1))
                    t = cpool.tile([128, BLK], f16, tag=f"h1_{d1c}", bufs=5)
                    nc.scalar.activation(t[:, :B], ps[:, :B], AF.Relu,
                                         bias=b1sb[:, d1c:d1c + 1])
                    h1.append(t)
                st_h1[b] = h1

            if 0 <= it - 1 < nblk:
                b, B = it - 1, blk_of(it - 1)
                h1 = st_h1[b]
                s = []
                for d2c in range(ND2):
                    pa = ppool.tile([128, BLK], f32, tag="pag", bufs=3)
                    for d1c in range(ND1):
                        lo = d1c * D2 + d2c * 128
                        nc.tensor.matmul(pa[:, :B], Wasb[:, lo:lo + 128],
                                         h1[d1c][:, :B],
                                         start=(d1c == 0), stop=(d1c == ND1 - 1))
                    at = cpool.tile([128, BLK], f16, tag=f"a_{d2c}", bufs=2)
                    nc.scalar.activation(at[:, :B], pa[:, :B], AF.Tanh,
                                         bias=basb[:, d2c:d2c + 1])

                    pg = ppool.tile([128, BLK], f32, tag="pag", bufs=3)
                    for d1c in range(ND1):
                        lo = d1c * D2 + d2c * 128
                        nc.tensor.matmul(pg[:, :B], Wbsb[:, lo:lo + 128],
                                         h1[d1c][:, :B],
                                         start=(d1c == 0), stop=(d1c == ND1 - 1))
                    gt = cpool.tile([128, BLK], f16, tag=f"g_{d2c}", bufs=2)
                    nc.scalar.activation(gt[:, :B], pg[:, :B], AF.Sigmoid,
                                         bias=bbsb[:, d2c:d2c + 1])

                    st = cpool.tile([128, BLK], f16, tag=f"s_{d2c}", bufs=3)
                    nc.vector.tensor_mul(st[:, :B], at[:, :B], gt[:, :B])
                    s.append(st)
                st_s[b] = s

            if 0 <= it - 2 < nblk:
                b = it - 2
                r0, B = blocks[b]
                s = st_s.pop(b)
                pA = ppool.tile([1, BLK], f32, tag="pA", bufs=2)
                for d2c in range(ND2):
                    nc.tensor.matmul(pA[:1, :B], Wcsb[:, d2c:d2c + 1],
                                     s[d2c][:, :B],
                                     start=(d2c == 0), stop=(d2c == ND2 - 1))
                nc.scalar.activation(A_sb[:1, r0:r0 + B], pA[:1, :B], AF.Identity,
                                     bias=bcsb[:1, 0:1])
                Et = cpool.tile([1, BLK], f16, tag="E", bufs=3)
                nc.scalar.activation(Et[:1, :B], pA[:1, :B], AF.Exp,
                                     bias=bcsb[:1, 0:1])
                st_E[b] = Et

            if 0 <= it - 3 < nblk:
                b, B = it - 3, blk_of(it - 3)
                h1 = st_h1.pop(b)
                Et = st_E.pop(b)
                # broadcast E to all 128 partitions via SWDGE DMA (0-step
                # free dim on the source) — keeps the PE stream pure matmul
                Es = cpool.tile([128, BLK], f16, tag="Es", bufs=2)
                src = (Et[0:1, :B].rearrange("p (a f) -> p a f", a=1)
                       .broadcast_to((1, 128, B)))
                nc.sync.dma_start(Es[:, :B], src)
                for d1c in range(ND1):
                    tmp = cpool.tile([128, BLK], f32, tag="pooltmp", bufs=2)
                    nc.vector.scalar_tensor_tensor(
                        out=tmp[:, :B], in0=h1[d1c][:, :B], scalar=1.0,
                        in1=Es[:, :B], op0=mult, op1=mult,
                        accum_out=Msum[d1c][:, b:b + 1])

        for d1c in range(ND1):
            nc.vector.reduce_sum(Macc[:, d1c:d1c + 1], Msum[d1c][:, :],
                                 axis=mybir.AxisListType.X)

        nc.sync.dma_start(A_out[:, :], A_sb[:1, :])
        nc.sync.dma_start(M_out[:, :], Macc[:, :])

    _split_multi_waits(nc)
    return nc


def _run_device(h):
    """Shard/cast/transpose h, run the SPMD kernel, return (A_raw[N], M_raw[512])
    plus the per-core weight inputs captured in _run_device.weights."""
    from concourse.bass_utils import run_bass_kernel_spmd

    if "nc" not in _prog_cache:
        _prog_cache["nc"] = _build_program()
    nc = _prog_cache["nc"]

    w = _run_device.weights
    in_maps = []
    for c in range(NCORES):
        shard = h[c * R:(c + 1) * R, :]
        hT_c = np.ascontiguousarray(shard.astype(np.float16).T)
        in_maps.append({"hT": hT_c, **w})

    res = run_bass_kernel_spmd(nc, in_maps, core_ids=list(range(NCORES)))
    A_raw = np.concatenate([res.results[c]["A_out"][0] for c in range(NCORES)])
    M_raw = np.zeros(D1, np.float64)
    for c in range(NCORES):
        M_raw += res.results[c]["M_out"].T.reshape(D1)
    return A_raw, M_raw


_run_device.weights = None


def kernel(h, W1, b1, Wa, ba, Wb, bb, Wc, bc, Wbag, bbag, Winst, binst, label):
    h = np.asarray(h, dtype=np.float32)
    W1 = np.asarray(W1, dtype=np.float32)
    b1 = np.asarray(b1, dtype=np.float32)
    Wa = np.asarray(Wa, dtype=np.float32)
    ba = np.asarray(ba, dtype=np.float32)
    Wb = np.asarray(Wb, dtype=np.float32)
    bb = np.asarray(bb, dtype=np.float32)
    Wc = np.asarray(Wc, dtype=np.float32)
    bc = np.asarray(bc, dtype=np.float32)
    Wbag = np.asarray(Wbag, dtype=np.float32)
    bbag = np.asarray(bbag, dtype=np.float32)
    Winst = np.asarray(Winst, dtype=np.float32)
    binst = np.asarray(binst, dtype=np.float32)

    def sbuf_image(W, nchunk):
        # [nchunk*128, F] -> [128, nchunk*F]: img[p, c*F+j] = W[c*128+p, j]
        F = W.shape[1]
        return np.ascontiguousarray(
            W.reshape(nchunk, 128, F).transpose(1, 0, 2).reshape(128, nchunk * F)
        ).astype(np.float16)

    _run_device.weights = {
        "W1f": sbuf_image(W1, NKC),
        "Waf": sbuf_image(Wa, ND1),
        "Wbf": sbuf_image(Wb, ND1),
        "Wcf": sbuf_image(Wc, ND2),
        "b1f": np.ascontiguousarray(b1.reshape(ND1, 128).T.astype(np.float32)),
        "baf": np.ascontiguousarray(ba.reshape(ND2, 128).T.astype(np.float32)),
        "bbf": np.ascontiguousarray(bb.reshape(ND2, 128).T.astype(np.float32)),
        "bcf": bc.reshape(1, 1).astype(np.float32),
    }

    A_raw_dev, M_raw = _run_device(h)  # [N] f32 (device), [512] f64 partials

    # --- bag branch (host fp32, negligible cost) ---
    Aexp = np.exp(A_raw_dev.astype(np.float64))
    Z = Aexp.sum()
    M = (M_raw / Z).astype(np.float32)                    # [512] pooled vector
    logits = (M @ Wbag + bbag).reshape(1, NCLS).astype(np.float32)
    lmax = logits.max(axis=1, keepdims=True)
    e = np.exp(logits - lmax)
    Y_prob = (e / e.sum(axis=1, keepdims=True)).astype(np.float32)
    Y_hat = np.argmax(logits, axis=1).astype(np.int32)

    # --- instance branch: exact fp32 refinement of candidates ---
    top_cand = np.argpartition(-A_raw_dev, NCAND - 1)[:NCAND]
    bot_cand = np.argpartition(A_raw_dev, NCAND - 1)[:NCAND]
    cand = np.unique(np.concatenate([top_cand, bot_cand]))

    h_sel = h[cand]                                        # [|cand|, 1024]
    h1_sel = np.maximum(h_sel @ W1 + b1, 0.0).astype(np.float32)
    a_sel = np.tanh(h1_sel @ Wa + ba)
    g_sel = 1.0 / (1.0 + np.exp(-(h1_sel @ Wb + bb)))
    A_sel = ((a_sel * g_sel).astype(np.float32) @ Wc + bc).reshape(-1).astype(np.float32)

    # top_k on softmax scores == top_k on A (softmax monotonic); lax.top_k
    # breaks ties toward the lower index.
    order_desc = np.lexsort((cand, -A_sel))
    order_asc = np.lexsort((cand, A_sel))
    top_rows = order_desc[:TOPK]
    bot_rows = order_asc[:TOPK]
    sel_rows = np.concatenate([top_rows, bot_rows])
    all_inst = h1_sel[sel_rows]                            # [2K, 512] exact fp32
    inst_logits = (all_inst @ Winst + binst).astype(np.float32)

    A_raw = A_raw_dev.reshape(1, N).astype(np.float32)
    return (logits, Y_prob, Y_hat, A_raw, inst_logits)


# revision 21
# speedup vs baseline: 1.0809x; 1.0087x over previous
"""Trainium2 Bass kernel for nn_CLAM_SB (gated-attention MIL, topk instance mining).

Strategy (8 NeuronCores, instance dim N=100000 sharded 12500 rows/core):

  Device (per core, fp16 matmuls / fp32 accumulation):
    h1^T = relu(W1^T @ h^T + b1)          [512, 12500]  (D1 on partitions)
    a^T  = tanh(Wa^T @ h1^T + ba)         [256, 12500]
    g^T  = sigmoid(Wb^T @ h1^T + bb)      [256, 12500]
    A    = Wc^T @ (a*g)^T + bc            [1, 12500]   -> output A_raw shard
    E    = exp(A)                          broadcast to 128 partitions via PE
    M_raw[d] = sum_r E_r * h1^T[d, r]      (fused DVE tensor_tensor_reduce)

  Host (numpy fp32):
    - shard + cast h to fp16, pre-transpose per core (so no on-device transpose)
    - A_raw = concat of shards; Z = sum exp(A_raw)
    - pooled M = (sum_c M_raw_c)/Z; logits/Y_prob/Y_hat from M @ Wbag
    - top-k: candidates = global top-64/bottom-64 of device A_raw, then the
      candidate rows are recomputed exactly in fp32 (tiny: 128 rows) and the
      final top-8/bottom-8 + inst_logits come from that exact recompute.
      Device noise (~1e-4) is far below the candidate margin, and the final
      selection/ordering matches the fp32 reference exactly.
"""

import numpy as np
from contextlib import ExitStack

# Problem constants (hardcoded per harness contract).
N, L, D1, D2, TOPK, NCLS = 100000, 1024, 512, 256, 8, 2
NCORES = 8
R = N // NCORES           # 12500 rows per core
BLK = 512                 # rows per block (matmul moving dim / PSUM bank)
NKC = L // 128            # 8 contraction chunks for h @ W1
ND1 = D1 // 128           # 4 D1 chunks
ND2 = D2 // 128           # 2 D2 chunks
NCAND = 64                # top/bottom candidates refined on host

_prog_cache = {}


# ---------------------------------------------------------------------------
# Wait-splitting post-pass: the walrus build in this container rejects
# instructions whose sync_info carries more than one wait ("Too many sync
# wait commands"). Tile freely emits multi-waits; rewrite every instruction
# with k>1 waits into k-1 preceding single-wait NOPs on the same engine.
# Per-engine program order makes this semantically identical.
# ---------------------------------------------------------------------------
def _split_multi_waits(nc):
    import bass_rust
    import concourse.mybir as mybir

    engine_attr = {
        mybir.EngineType.PE: "tensor",
        mybir.EngineType.DVE: "vector",
        mybir.EngineType.Activation: "scalar",
        mybir.EngineType.Pool: "gpsimd",
        mybir.EngineType.SP: "sync",
    }

    def make_wait_nop(engine, wait):
        eng = getattr(nc, engine_attr[engine])
        inst = eng.nop(nofuse=True).ins
        for fn in nc.m.functions:
            for bb in fn.blocks:
                if inst in bb.instructions:
                    bb.instructions.remove(inst)
        inst.sync_info = bass_rust.SyncInfo(on_wait=[wait], on_update=[])
        return inst

    for fn in nc.m.functions:
        for bb in fn.blocks:
            new_insts = []
            for inst in bb.instructions:
                si = inst.sync_info
                if si is not None and si.on_wait and len(si.on_wait) > 1:
                    waits = list(si.on_wait)
                    for w in waits[:-1]:
                        new_insts.append(make_wait_nop(inst.engine, w))
                    inst.sync_info = bass_rust.SyncInfo(
                        on_wait=[waits[-1]], on_update=list(si.on_update or [])
                    )
                new_insts.append(inst)
            bb.instructions[:] = new_insts


def _build_program():
    """Build the per-core SPMD Bass program (same NEFF for all 8 cores)."""
    import concourse.bass as bass
    import concourse.tile as tile
    import concourse.mybir as mybir

    f16, f32 = mybir.dt.float16, mybir.dt.float32
    mult, add = mybir.AluOpType.mult, mybir.AluOpType.add
    AF = mybir.ActivationFunctionType

    nc = bass.Bass("TRN2", debug=False)

    hT = nc.dram_tensor("hT", [L, R], f16, kind="ExternalInput").ap()
    # weights arrive pre-arranged as their SBUF images (one contiguous,
    # descriptor-efficient DMA each): W1f[p, kc*D1+j] = W1[kc*128+p, j] etc.
    W1f = nc.dram_tensor("W1f", [128, NKC * D1], f16, kind="ExternalInput").ap()
    Waf = nc.dram_tensor("Waf", [128, ND1 * D2], f16, kind="ExternalInput").ap()
    Wbf = nc.dram_tensor("Wbf", [128, ND1 * D2], f16, kind="ExternalInput").ap()
    Wcf = nc.dram_tensor("Wcf", [128, ND2], f16, kind="ExternalInput").ap()
    b1f = nc.dram_tensor("b1f", [128, ND1], f32, kind="ExternalInput").ap()
    baf = nc.dram_tensor("baf", [128, ND2], f32, kind="ExternalInput").ap()
    bbf = nc.dram_tensor("bbf", [128, ND2], f32, kind="ExternalInput").ap()
    bcf = nc.dram_tensor("bcf", [1, 1], f32, kind="ExternalInput").ap()

    A_out = nc.dram_tensor("A_out", [1, R], f32, kind="ExternalOutput").ap()
    M_out = nc.dram_tensor("M_out", [128, ND1], f32, kind="ExternalOutput").ap()

    # Block schedule: 512-row blocks; h^T DMAs grouped in quads of blocks
    # (4KB partition lines for descriptor efficiency), prefetched one group
    # ahead of compute.
    blocks = []                                          # (r0, B)
    r = 0
    while r < R:
        B = min(BLK, R - r)
        blocks.append((r, B))
        r += B
    nblk = len(blocks)
    groups = []                                          # group -> block idxs
    b = 0
    while b < nblk:
        groups.append([bi for bi in range(b, min(b + 4, nblk))
                       if blocks[bi][1] == BLK or bi == b])
        b = groups[-1][-1] + 1
    ngrp = len(groups)
    gidx = {}
    goff = {}
    for g, bs in enumerate(groups):
        for j, bi in enumerate(bs):
            gidx[bi] = g
            goff[bi] = blocks[bi][0] - blocks[bs[0]][0]

    with tile.TileContext(nc) as tc, ExitStack() as ctx:
        wpool = ctx.enter_context(tc.tile_pool(name="weights", bufs=1))
        hpool = ctx.enter_context(tc.tile_pool(name="ht", bufs=1))
        cpool = ctx.enter_context(tc.tile_pool(name="compute", bufs=1))
        ppool = ctx.enter_context(tc.tile_pool(name="psum", bufs=1, space="PSUM"))

        # --- persistent weights/biases in SBUF ---
        # W1 tiles are interleaved with the first h^T block so the PE's first
        # matmul dependencies finish loading as early as possible; everything
        # only needed from pipeline stage 2 onward loads afterwards.
        ht_groups = {}   # g -> list of NKC tiles [128, 2*BLK]

        def load_group(g):
            bs = groups[g]
            r0 = blocks[bs[0]][0]
            w = blocks[bs[-1]][0] + blocks[bs[-1]][1] - r0
            tiles = []
            for kc in range(NKC):
                t = hpool.tile([128, 4 * BLK], f16, tag=f"ht{kc}", bufs=2)
                nc.sync.dma_start(
                    t[:, :w], hT[kc * 128:(kc + 1) * 128, r0:r0 + w])
                tiles.append(t)
            ht_groups[g] = tiles

        # W1 (one contiguous 8KB-per-line DMA) then the first two h^T groups.
        W1sb = wpool.tile([128, NKC * D1], f16)          # [k, kc*512 + j]
        nc.sync.dma_start(W1sb[:], W1f[:, :])
        load_group(0)
        Wasb = wpool.tile([128, ND1 * D2], f16)          # [k, d1c*256 + j]
        nc.sync.dma_start(Wasb[:], Waf[:, :])
        Wbsb = wpool.tile([128, ND1 * D2], f16)
        nc.sync.dma_start(Wbsb[:], Wbf[:, :])
        Wcsb = wpool.tile([128, ND2], f16)               # [k, d2c]
        nc.sync.dma_start(Wcsb[:], Wcf[:, :])
        b1sb = wpool.tile([128, ND1], f32)
        nc.sync.dma_start(b1sb[:], b1f[:, :])
        basb = wpool.tile([128, ND2], f32)
        nc.sync.dma_start(basb[:], baf[:, :])
        bbsb = wpool.tile([128, ND2], f32)
        nc.sync.dma_start(bbsb[:], bbf[:, :])
        bcsb = wpool.tile([1, 1], f32)
        nc.sync.dma_start(bcsb[:], bcf[:, :])

        # --- persistent accumulators / staged outputs ---
        A_sb = wpool.tile([1, R], f32)
        Macc = wpool.tile([128, ND1], f32)
        # per-block pooled partial sums; reduced into Macc at the end
        Msum = [wpool.tile([128, nblk], f32, name=f"Msum{d1c}")
                for d1c in range(ND1)]

        # 4-stage software pipeline, one block of skew between stages, so the
        # PE instruction stream never waits on ACT/DVE results of the same
        # block: stage1(b)=load+h1, stage2(b-1)=a/g/s, stage3(b-2)=A+exp,
        # stage4(b-3)=E broadcast + pooled partial.
        st_h1 = {}   # b -> list of 4 h1^T tiles
        st_s = {}    # b -> list of 2 s tiles
        st_E = {}    # b -> Et tile

        def blk_of(b):
            return blocks[b][1]

        for it in range(nblk + 3):
            if it < nblk:
                b, B = it, blk_of(it)
                g, off = gidx[b], goff[b]
                if b == groups[g][0] and g + 1 < ngrp and g + 1 not in ht_groups:
                    load_group(g + 1)
                ht = ht_groups[g]
                h1 = []
                for d1c in range(ND1):
                    ps = ppool.tile([128, BLK], f32, tag="ph1", bufs=3)
                    for kc in range(NKC):
                        lo = kc * D1 + d1c * 128
                        nc.tensor.matmul(ps[:, :B], W1sb[:, lo:lo + 128],
                                         ht[kc][:, off:off + B],
                                         start=(kc == 0), stop=(kc == NKC - 1))
                    t = cpool.tile([128, BLK], f16, tag=f"h1_{d1c}", bufs=5)
                    nc.scalar.activation(t[:, :B], ps[:, :B], AF.Relu,
                                         bias=b1sb[:, d1c:d1c + 1])
                    h1.append(t)
                st_h1[b] = h1

            if 0 <= it - 1 < nblk:
                b, B = it - 1, blk_of(it - 1)
                h1 = st_h1[b]
                s = []
                for d2c in range(ND2):
                    pa = ppool.tile([128, BLK], f32, tag="pag", bufs=3)
                    for d1c in range(ND1):
                        lo = d1c * D2 + d2c * 128
                        nc.tensor.matmul(pa[:, :B], Wasb[:, lo:lo + 128],
                                         h1[d1c][:, :B],
                                         start=(d1c == 0), stop=(d1c == ND1 - 1))
                    at = cpool.tile([128, BLK], f16, tag=f"a_{d2c}", bufs=2)
                    nc.scalar.activation(at[:, :B], pa[:, :B], AF.Tanh,
                                         bias=basb[:, d2c:d2c + 1])

                    pg = ppool.tile([128, BLK], f32, tag="pag", bufs=3)
                    for d1c in range(ND1):
                        lo = d1c * D2 + d2c * 128
                        nc.tensor.matmul(pg[:, :B], Wbsb[:, lo:lo + 128],
                                         h1[d1c][:, :B],
                                         start=(d1c == 0), stop=(d1c == ND1 - 1))
                    gt = cpool.tile([128, BLK], f16, tag=f"g_{d2c}", bufs=2)
                    nc.scalar.activation(gt[:, :B], pg[:, :B], AF.Sigmoid,
                                         bias=bbsb[:, d2c:d2c + 1])

                    st = cpool.tile([128, BLK], f16, tag=f"s_{d2c}", bufs=3)
                    nc.vector.tensor_mul(st[:, :B], at[:, :B], gt[:, :B])
                    s.append(st)
                st_s[b] = s

            if 0 <= it - 2 < nblk:
                b = it - 2
                r0, B = blocks[b]
                s = st_s.pop(b)
                pA = ppool.tile([1, BLK], f32, tag="pA", bufs=2)
                for d2c in range(ND2):
                    nc.tensor.matmul(pA[:1, :B], Wcsb[:, d2c:d2c + 1],
                                     s[d2c][:, :B],
                                     start=(d2c == 0), stop=(d2c == ND2 - 1))
                nc.scalar.activation(A_sb[:1, r0:r0 + B], pA[:1, :B], AF.Identity,
                                     bias=bcsb[:1, 0:1])
                Et = cpool.tile([1, BLK], f16, tag="E", bufs=3)
                nc.scalar.activation(Et[:1, :B], pA[:1, :B], AF.Exp,
                                     bias=bcsb[:1, 0:1])
                st_E[b] = Et

            if 0 <= it - 3 < nblk:
                b, B = it - 3, blk_of(it - 3)
                h1 = st_h1.pop(b)
                Et = st_E.pop(b)
                # broadcast E to all 128 partitions via SWDGE DMA (0-step
                # free dim on the source) — keeps the PE stream pure matmul
                Es = cpool.tile([128, BLK], f16, tag="Es", bufs=2)
                src = (Et[0:1, :B].rearrange("p (a f) -> p a f", a=1)
                       .broadcast_to((1, 128, B)))
                nc.sync.dma_start(Es[:, :B], src)
                for d1c in range(ND1):
                    tmp = cpool.tile([128, BLK], f32, tag="pooltmp", bufs=2)
                    nc.vector.scalar_tensor_tensor(
                        out=tmp[:, :B], in0=h1[d1c][:, :B], scalar=1.0,
                        in1=Es[:, :B], op0=mult, op1=mult,
                        accum_out=Msum[d1c][:, b:b + 1])

        for d1c in range(ND1):
            nc.vector.reduce_sum(Macc[:, d1c:d1c + 1], Msum[d1c][:, :],
                                 axis=mybir.AxisListType.X)

        nc.sync.dma_start(A_out[:, :], A_sb[:1, :])
        nc.sync.dma_start(M_out[:, :], Macc[:, :])

    _split_multi_waits(nc)
    return nc


def _run_device(h):
    """Shard/cast/transpose h, run the SPMD kernel, return (A_raw[N], M_raw[512])
    plus the per-core weight inputs captured in _run_device.weights."""
    from concourse.bass_utils import run_bass_kernel_spmd

    if "nc" not in _prog_cache:
        _prog_cache["nc"] = _build_program()
    nc = _prog_cache["nc"]

    w = _run_device.weights
    in_maps = []
    for c in range(NCORES):
        shard = h[c * R:(c + 1) * R, :]
        hT_c = np.ascontiguousarray(shard.astype(np.float16).T)
        in_maps.append({"hT": hT_c, **w})

    res = run_bass_kernel_spmd(nc, in_maps, core_ids=list(range(NCORES)))
    A_raw = np.concatenate([res.results[c]["A_out"][0] for c in range(NCORES)])
    M_raw = np.zeros(D1, np.float64)
    for c in range(NCORES):
        M_raw += res.results[c]["M_out"].T.reshape(D1)
    return A_raw, M_raw


_run_device.weights = None


def kernel(h, W1, b1, Wa, ba, Wb, bb, Wc, bc, Wbag, bbag, Winst, binst, label):
    h = np.asarray(h, dtype=np.float32)
    W1 = np.asarray(W1, dtype=np.float32)
    b1 = np.asarray(b1, dtype=np.float32)
    Wa = np.asarray(Wa, dtype=np.float32)
    ba = np.asarray(ba, dtype=np.float32)
    Wb = np.asarray(Wb, dtype=np.float32)
    bb = np.asarray(bb, dtype=np.float32)
    Wc = np.asarray(Wc, dtype=np.float32)
    bc = np.asarray(bc, dtype=np.float32)
    Wbag = np.asarray(Wbag, dtype=np.float32)
    bbag = np.asarray(bbag, dtype=np.float32)
    Winst = np.asarray(Winst, dtype=np.float32)
    binst = np.asarray(binst, dtype=np.float32)

    def sbuf_image(W, nchunk):
        # [nchunk*128, F] -> [128, nchunk*F]: img[p, c*F+j] = W[c*128+p, j]
        F = W.shape[1]
        return np.ascontiguousarray(
            W.reshape(nchunk, 128, F).transpose(1, 0, 2).reshape(128, nchunk * F)
        ).astype(np.float16)

    _run_device.weights = {
        "W1f": sbuf_image(W1, NKC),
        "Waf": sbuf_image(Wa, ND1),
        "Wbf": sbuf_image(Wb, ND1),
        "Wcf": sbuf_image(Wc, ND2),
        "b1f": np.ascontiguousarray(b1.reshape(ND1, 128).T.astype(np.float32)),
        "baf": np.ascontiguousarray(ba.reshape(ND2, 128).T.astype(np.float32)),
        "bbf": np.ascontiguousarray(bb.reshape(ND2, 128).T.astype(np.float32)),
        "bcf": bc.reshape(1, 1).astype(np.float32),
    }

    A_raw_dev, M_raw = _run_device(h)  # [N] f32 (device), [512] f64 partials

    # --- bag branch (host fp32, negligible cost) ---
    Aexp = np.exp(A_raw_dev.astype(np.float64))
    Z = Aexp.sum()
    M = (M_raw / Z).astype(np.float32)                    # [512] pooled vector
    logits = (M @ Wbag + bbag).reshape(1, NCLS).astype(np.float32)
    lmax = logits.max(axis=1, keepdims=True)
    e = np.exp(logits - lmax)
    Y_prob = (e / e.sum(axis=1, keepdims=True)).astype(np.float32)
    Y_hat = np.argmax(logits, axis=1).astype(np.int32)

    # --- instance branch: exact fp32 refinement of candidates ---
    top_cand = np.argpartition(-A_raw_dev, NCAND - 1)[:NCAND]
    bot_cand = np.argpartition(A_raw_dev, NCAND - 1)[:NCAND]
    cand = np.unique(np.concatenate([top_cand, bot_cand]))

    h_sel = h[cand]                                        # [|cand|, 1024]
    h1_sel = np.maximum(h_sel @ W1 + b1, 0.0).astype(np.float32)
    a_sel = np.tanh(h1_sel @ Wa + ba)
    g_sel = 1.0 / (1.0 + np.exp(-(h1_sel @ Wb + bb)))
    A_sel = ((a_sel * g_sel).astype(np.float32) @ Wc + bc).reshape(-1).astype(np.float32)

    # top_k on softmax scores == top_k on A (softmax monotonic); lax.top_k
    # breaks ties toward the lower index.
    order_desc = np.lexsort((cand, -A_sel))
    order_asc = np.lexsort((cand, A_sel))
    top_rows = order_desc[:TOPK]
    bot_rows = order_asc[:TOPK]
    sel_rows = np.concatenate([top_rows, bot_rows])
    all_inst = h1_sel[sel_rows]                            # [2K, 512] exact fp32
    inst_logits = (all_inst @ Winst + binst).astype(np.float32)

    A_raw = A_raw_dev.reshape(1, N).astype(np.float32)
    return (logits, Y_prob, Y_hat, A_raw, inst_logits)


# revision 22
# speedup vs baseline: 1.0833x; 1.0022x over previous
"""Trainium2 Bass kernel for nn_CLAM_SB (gated-attention MIL, topk instance mining).

Strategy (8 NeuronCores, instance dim N=100000 sharded 12500 rows/core):

  Device (per core, fp16 matmuls / fp32 accumulation):
    h1^T = relu(W1^T @ h^T + b1)          [512, 12500]  (D1 on partitions)
    a^T  = tanh(Wa^T @ h1^T + ba)         [256, 12500]
    g^T  = sigmoid(Wb^T @ h1^T + bb)      [256, 12500]
    A    = Wc^T @ (a*g)^T + bc            [1, 12500]   -> output A_raw shard
    E    = exp(A)                          broadcast to 128 partitions via PE
    M_raw[d] = sum_r E_r * h1^T[d, r]      (fused DVE tensor_tensor_reduce)

  Host (numpy fp32):
    - shard + cast h to fp16, pre-transpose per core (so no on-device transpose)
    - A_raw = concat of shards; Z = sum exp(A_raw)
    - pooled M = (sum_c M_raw_c)/Z; logits/Y_prob/Y_hat from M @ Wbag
    - top-k: candidates = global top-64/bottom-64 of device A_raw, then the
      candidate rows are recomputed exactly in fp32 (tiny: 128 rows) and the
      final top-8/bottom-8 + inst_logits come from that exact recompute.
      Device noise (~1e-4) is far below the candidate margin, and the final
      selection/ordering matches the fp32 reference exactly.
"""

import numpy as np
from contextlib import ExitStack

# Problem constants (hardcoded per harness contract).
N, L, D1, D2, TOPK, NCLS = 100000, 1024, 512, 256, 8, 2
NCORES = 8
R = N // NCORES           # 12500 rows per core
BLK = 512                 # rows per block (matmul moving dim / PSUM bank)
NKC = L // 128            # 8 contraction chunks for h @ W1
ND1 = D1 // 128           # 4 D1 chunks
ND2 = D2 // 128           # 2 D2 chunks
NCAND = 64                # top/bottom candidates refined on host

_prog_cache = {}


# ---------------------------------------------------------------------------
# Wait-splitting post-pass: the walrus build in this container rejects
# instructions whose sync_info carries more than one wait ("Too many sync
# wait commands"). Tile freely emits multi-waits; rewrite every instruction
# with k>1 waits into k-1 preceding single-wait NOPs on the same engine.
# Per-engine program order makes this semantically identical.
# ---------------------------------------------------------------------------
def _split_multi_waits(nc):
    import bass_rust
    import concourse.mybir as mybir

    engine_attr = {
        mybir.EngineType.PE: "tensor",
        mybir.EngineType.DVE: "vector",
        mybir.EngineType.Activation: "scalar",
        mybir.EngineType.Pool: "gpsimd",
        mybir.EngineType.SP: "sync",
    }

    def make_wait_nop(engine, wait):
        eng = getattr(nc, engine_attr[engine])
        inst = eng.nop(nofuse=True).ins
        for fn in nc.m.functions:
            for bb in fn.blocks:
                if inst in bb.instructions:
                    bb.instructions.remove(inst)
        inst.sync_info = bass_rust.SyncInfo(on_wait=[wait], on_update=[])
        return inst

    for fn in nc.m.functions:
        for bb in fn.blocks:
            new_insts = []
            for inst in bb.instructions:
                si = inst.sync_info
                if si is not None and si.on_wait and len(si.on_wait) > 1:
                    waits = list(si.on_wait)
                    for w in waits[:-1]:
                        new_insts.append(make_wait_nop(inst.engine, w))
                    inst.sync_info = bass_rust.SyncInfo(
                        on_wait=[waits[-1]], on_update=list(si.on_update or [])
                    )
                new_insts.append(inst)
            bb.instructions[:] = new_insts


def _build_program():
    """Build the per-core SPMD Bass program (same NEFF for all 8 cores)."""
    import concourse.bass as bass
    import concourse.tile as tile
    import concourse.mybir as mybir

    f16, f32 = mybir.dt.float16, mybir.dt.float32
    mult, add = mybir.AluOpType.mult, mybir.AluOpType.add
    AF = mybir.ActivationFunctionType

    nc = bass.Bass("TRN2", debug=False)

    hT = nc.dram_tensor("hT", [L, R], f16, kind="ExternalInput").ap()
    # weights arrive pre-arranged as their SBUF images (one contiguous,
    # descriptor-efficient DMA each): W1f[p, kc*D1+j] = W1[kc*128+p, j] etc.
    W1f = nc.dram_tensor("W1f", [128, NKC * D1], f16, kind="ExternalInput").ap()
    Waf = nc.dram_tensor("Waf", [128, ND1 * D2], f16, kind="ExternalInput").ap()
    Wbf = nc.dram_tensor("Wbf", [128, ND1 * D2], f16, kind="ExternalInput").ap()
    Wcf = nc.dram_tensor("Wcf", [128, ND2], f16, kind="ExternalInput").ap()
    b1f = nc.dram_tensor("b1f", [128, ND1], f32, kind="ExternalInput").ap()
    baf = nc.dram_tensor("baf", [128, ND2], f32, kind="ExternalInput").ap()
    bbf = nc.dram_tensor("bbf", [128, ND2], f32, kind="ExternalInput").ap()
    bcf = nc.dram_tensor("bcf", [1, 1], f32, kind="ExternalInput").ap()

    A_out = nc.dram_tensor("A_out", [1, R], f32, kind="ExternalOutput").ap()
    M_out = nc.dram_tensor("M_out", [128, ND1], f32, kind="ExternalOutput").ap()

    # Block schedule: 512-row blocks; h^T DMAs grouped in quads of blocks
    # (4KB partition lines for descriptor efficiency), prefetched one group
    # ahead of compute.
    blocks = []                                          # (r0, B)
    r = 0
    while r < R:
        B = min(BLK, R - r)
        blocks.append((r, B))
        r += B
    nblk = len(blocks)
    groups = []                                          # group -> block idxs
    b = 0
    while b < nblk:
        groups.append([bi for bi in range(b, min(b + 4, nblk))
                       if blocks[bi][1] == BLK or bi == b])
        b = groups[-1][-1] + 1
    ngrp = len(groups)
    gidx = {}
    goff = {}
    for g, bs in enumerate(groups):
        for j, bi in enumerate(bs):
            gidx[bi] = g
            goff[bi] = blocks[bi][0] - blocks[bs[0]][0]

    with tile.TileContext(nc) as tc, ExitStack() as ctx:
        wpool = ctx.enter_context(tc.tile_pool(name="weights", bufs=1))
        hpool = ctx.enter_context(tc.tile_pool(name="ht", bufs=1))
        cpool = ctx.enter_context(tc.tile_pool(name="compute", bufs=1))
        ppool = ctx.enter_context(tc.tile_pool(name="psum", bufs=1, space="PSUM"))

        # --- persistent weights/biases in SBUF ---
        # W1 tiles are interleaved with the first h^T block so the PE's first
        # matmul dependencies finish loading as early as possible; everything
        # only needed from pipeline stage 2 onward loads afterwards.
        ht_groups = {}   # g -> list of NKC tiles [128, 2*BLK]

        def load_group(g):
            bs = groups[g]
            r0 = blocks[bs[0]][0]
            w = blocks[bs[-1]][0] + blocks[bs[-1]][1] - r0
            tiles = []
            for kc in range(NKC):
                t = hpool.tile([128, 4 * BLK], f16, tag=f"ht{kc}", bufs=2)
                nc.sync.dma_start(
                    t[:, :w], hT[kc * 128:(kc + 1) * 128, r0:r0 + w])
                tiles.append(t)
            ht_groups[g] = tiles

        # W1 (one contiguous 8KB-per-line DMA) then the first two h^T groups.
        W1sb = wpool.tile([128, NKC * D1], f16)          # [k, kc*512 + j]
        nc.sync.dma_start(W1sb[:], W1f[:, :])
        load_group(0)
        Wasb = wpool.tile([128, ND1 * D2], f16)          # [k, d1c*256 + j]
        nc.sync.dma_start(Wasb[:], Waf[:, :])
        Wbsb = wpool.tile([128, ND1 * D2], f16)
        nc.sync.dma_start(Wbsb[:], Wbf[:, :])
        Wcsb = wpool.tile([128, ND2], f16)               # [k, d2c]
        nc.sync.dma_start(Wcsb[:], Wcf[:, :])
        b1sb = wpool.tile([128, ND1], f32)
        nc.sync.dma_start(b1sb[:], b1f[:, :])
        basb = wpool.tile([128, ND2], f32)
        nc.sync.dma_start(basb[:], baf[:, :])
        bbsb = wpool.tile([128, ND2], f32)
        nc.sync.dma_start(bbsb[:], bbf[:, :])
        bcsb = wpool.tile([1, 1], f32)
        nc.sync.dma_start(bcsb[:], bcf[:, :])

        # --- persistent accumulators / staged outputs ---
        A_sb = wpool.tile([1, R], f32)
        Macc = wpool.tile([128, ND1], f32)
        # per-block pooled partial sums; reduced into Macc at the end
        Msum = [wpool.tile([128, nblk], f32, name=f"Msum{d1c}")
                for d1c in range(ND1)]

        # 4-stage software pipeline, one block of skew between stages, so the
        # PE instruction stream never waits on ACT/DVE results of the same
        # block: stage1(b)=load+h1, stage2(b-1)=a/g/s, stage3(b-2)=A+exp,
        # stage4(b-3)=E broadcast + pooled partial.
        st_h1 = {}   # b -> list of 4 h1^T tiles
        st_s = {}    # b -> list of 2 s tiles
        st_E = {}    # b -> Et tile

        def blk_of(b):
            return blocks[b][1]

        for it in range(nblk + 3):
            if it < nblk:
                b, B = it, blk_of(it)
                g, off = gidx[b], goff[b]
                if b == groups[g][0] and g + 1 < ngrp and g + 1 not in ht_groups:
                    load_group(g + 1)
                ht = ht_groups[g]
                h1 = []
                for d1c in range(ND1):
                    ps = ppool.tile([128, BLK], f32, tag="ph1", bufs=3)
                    for kc in range(NKC):
                        lo = kc * D1 + d1c * 128
                        nc.tensor.matmul(ps[:, :B], W1sb[:, lo:lo + 128],
                                         ht[kc][:, off:off + B],
                                         start=(kc == 0), stop=(kc == NKC - 1))
                    t = cpool.tile([128, BLK], f16, tag=f"h1_{d1c}", bufs=5)
                    nc.scalar.activation(t[:, :B], ps[:, :B], AF.Relu,
                                         bias=b1sb[:, d1c:d1c + 1])
                    h1.append(t)
                st_h1[b] = h1

            if 0 <= it - 1 < nblk:
                b, B = it - 1, blk_of(it - 1)
                h1 = st_h1[b]
                s = []
                for d2c in range(ND2):
                    pa = ppool.tile([128, BLK], f32, tag="pag", bufs=3)
                    for d1c in range(ND1):
                        lo = d1c * D2 + d2c * 128
                        nc.tensor.matmul(pa[:, :B], Wasb[:, lo:lo + 128],
                                         h1[d1c][:, :B],
                                         start=(d1c == 0), stop=(d1c == ND1 - 1))
                    at = cpool.tile([128, BLK], f16, tag=f"a_{d2c}", bufs=2)
                    nc.scalar.activation(at[:, :B], pa[:, :B], AF.Tanh,
                                         bias=basb[:, d2c:d2c + 1])

                    pg = ppool.tile([128, BLK], f32, tag="pag", bufs=3)
                    for d1c in range(ND1):
                        lo = d1c * D2 + d2c * 128
                        nc.tensor.matmul(pg[:, :B], Wbsb[:, lo:lo + 128],
                                         h1[d1c][:, :B],
                                         start=(d1c == 0), stop=(d1c == ND1 - 1))
                    gt = cpool.tile([128, BLK], f16, tag=f"g_{d2c}", bufs=2)
                    nc.scalar.activation(gt[:, :B], pg[:, :B], AF.Sigmoid,
                                         bias=bbsb[:, d2c:d2c + 1])

                    st = cpool.tile([128, BLK], f16, tag=f"s_{d2c}", bufs=3)
                    nc.vector.tensor_mul(st[:, :B], at[:, :B], gt[:, :B])
                    s.append(st)
                st_s[b] = s

            if 0 <= it - 2 < nblk:
                b = it - 2
                r0, B = blocks[b]
                s = st_s.pop(b)
                pA = ppool.tile([1, BLK], f32, tag="pA", bufs=2)
                for d2c in range(ND2):
                    nc.tensor.matmul(pA[:1, :B], Wcsb[:, d2c:d2c + 1],
                                     s[d2c][:, :B],
                                     start=(d2c == 0), stop=(d2c == ND2 - 1))
                nc.scalar.activation(A_sb[:1, r0:r0 + B], pA[:1, :B], AF.Identity,
                                     bias=bcsb[:1, 0:1])
                Et = cpool.tile([1, BLK], f16, tag="E", bufs=3)
                nc.scalar.activation(Et[:1, :B], pA[:1, :B], AF.Exp,
                                     bias=bcsb[:1, 0:1])
                st_E[b] = Et

            if 0 <= it - 3 < nblk:
                b, B = it - 3, blk_of(it - 3)
                h1 = st_h1.pop(b)
                Et = st_E.pop(b)
                # broadcast E to all 128 partitions via SWDGE DMA (0-step
                # free dim on the source) — keeps the PE stream pure matmul
                Es = cpool.tile([128, BLK], f16, tag="Es", bufs=2)
                src = (Et[0:1, :B].rearrange("p (a f) -> p a f", a=1)
                       .broadcast_to((1, 128, B)))
                nc.sync.dma_start(Es[:, :B], src)
                for d1c in range(ND1):
                    tmp = cpool.tile([128, BLK], f32, tag="pooltmp", bufs=2)
                    nc.vector.scalar_tensor_tensor(
                        out=tmp[:, :B], in0=h1[d1c][:, :B], scalar=1.0,
                        in1=Es[:, :B], op0=mult, op1=mult,
                        accum_out=Msum[d1c][:, b:b + 1])

        for d1c in range(ND1):
            nc.vector.reduce_sum(Macc[:, d1c:d1c + 1], Msum[d1c][:, :],
                                 axis=mybir.AxisListType.X)

        nc.sync.dma_start(A_out[:, :], A_sb[:1, :])
        nc.sync.dma_start(M_out[:, :], Macc[:, :])

    _split_multi_waits(nc)
    return nc


def _run_device(h):
    """Shard/cast/transpose h, run the SPMD kernel, return (A_raw[N], M_raw[512])
    plus the per-core weight inputs captured in _run_device.weights."""
    from concourse.bass_utils import run_bass_kernel_spmd

    if "nc" not in _prog_cache:
        _prog_cache["nc"] = _build_program()
    nc = _prog_cache["nc"]

    w = _run_device.weights
    in_maps = []
    for c in range(NCORES):
        shard = h[c * R:(c + 1) * R, :]
        hT_c = np.ascontiguousarray(shard.astype(np.float16).T)
        in_maps.append({"hT": hT_c, **w})

    try:
        res = run_bass_kernel_spmd(nc, in_maps, core_ids=list(range(NCORES)))
    except Exception:
        # The axon-tunneled runtime occasionally reports the accelerator as
        # unrecoverable right after a profiled session; a retry on a fresh
        # dispatch clears it.
        import time
        time.sleep(2.0)
        res = run_bass_kernel_spmd(nc, in_maps, core_ids=list(range(NCORES)))
    A_raw = np.concatenate([res.results[c]["A_out"][0] for c in range(NCORES)])
    M_raw = np.zeros(D1, np.float64)
    for c in range(NCORES):
        M_raw += res.results[c]["M_out"].T.reshape(D1)
    return A_raw, M_raw


_run_device.weights = None


def kernel(h, W1, b1, Wa, ba, Wb, bb, Wc, bc, Wbag, bbag, Winst, binst, label):
    h = np.asarray(h, dtype=np.float32)
    W1 = np.asarray(W1, dtype=np.float32)
    b1 = np.asarray(b1, dtype=np.float32)
    Wa = np.asarray(Wa, dtype=np.float32)
    ba = np.asarray(ba, dtype=np.float32)
    Wb = np.asarray(Wb, dtype=np.float32)
    bb = np.asarray(bb, dtype=np.float32)
    Wc = np.asarray(Wc, dtype=np.float32)
    bc = np.asarray(bc, dtype=np.float32)
    Wbag = np.asarray(Wbag, dtype=np.float32)
    bbag = np.asarray(bbag, dtype=np.float32)
    Winst = np.asarray(Winst, dtype=np.float32)
    binst = np.asarray(binst, dtype=np.float32)

    def sbuf_image(W, nchunk):
        # [nchunk*128, F] -> [128, nchunk*F]: img[p, c*F+j] = W[c*128+p, j]
        F = W.shape[1]
        return np.ascontiguousarray(
            W.reshape(nchunk, 128, F).transpose(1, 0, 2).reshape(128, nchunk * F)
        ).astype(np.float16)

    _run_device.weights = {
        "W1f": sbuf_image(W1, NKC),
        "Waf": sbuf_image(Wa, ND1),
        "Wbf": sbuf_image(Wb, ND1),
        "Wcf": sbuf_image(Wc, ND2),
        "b1f": np.ascontiguousarray(b1.reshape(ND1, 128).T.astype(np.float32)),
        "baf": np.ascontiguousarray(ba.reshape(ND2, 128).T.astype(np.float32)),
        "bbf": np.ascontiguousarray(bb.reshape(ND2, 128).T.astype(np.float32)),
        "bcf": bc.reshape(1, 1).astype(np.float32),
    }

    A_raw_dev, M_raw = _run_device(h)  # [N] f32 (device), [512] f64 partials

    # --- bag branch (host fp32, negligible cost) ---
    Aexp = np.exp(A_raw_dev.astype(np.float64))
    Z = Aexp.sum()
    M = (M_raw / Z).astype(np.float32)                    # [512] pooled vector
    logits = (M @ Wbag + bbag).reshape(1, NCLS).astype(np.float32)
    lmax = logits.max(axis=1, keepdims=True)
    e = np.exp(logits - lmax)
    Y_prob = (e / e.sum(axis=1, keepdims=True)).astype(np.float32)
    Y_hat = np.argmax(logits, axis=1).astype(np.int32)

    # --- instance branch: exact fp32 refinement of candidates ---
    top_cand = np.argpartition(-A_raw_dev, NCAND - 1)[:NCAND]
    bot_cand = np.argpartition(A_raw_dev, NCAND - 1)[:NCAND]
    cand = np.unique(np.concatenate([top_cand, bot_cand]))

    h_sel = h[cand]                                        # [|cand|, 1024]
    h1_sel = np.maximum(h_sel @ W1 + b1, 0.0).astype(np.float32)
    a_sel = np.tanh(h1_sel @ Wa + ba)
    g_sel = 1.0 / (1.0 + np.exp(-(h1_sel @ Wb + bb)))
    A_sel = ((a_sel * g_sel).astype(np.float32) @ Wc + bc).reshape(-1).astype(np.float32)

    # top_k on softmax scores == top_k on A (softmax monotonic); lax.top_k
    # breaks ties toward the lower index.
    order_desc = np.lexsort((cand, -A_sel))
    order_asc = np.lexsort((cand, A_sel))
    top_rows = order_desc[:TOPK]
    bot_rows = order_asc[:TOPK]
    sel_rows = np.concatenate([top_rows, bot_rows])
    all_inst = h1_sel[sel_rows]                            # [2K, 512] exact fp32
    inst_logits = (all_inst @ Winst + binst).astype(np.float32)

    A_raw = A_raw_dev.reshape(1, N).astype(np.float32)
    return (logits, Y_prob, Y_hat, A_raw, inst_logits)


# revision 23
# speedup vs baseline: 1.0927x; 1.0087x over previous
"""Trainium2 Bass kernel for nn_CLAM_SB (gated-attention MIL, topk instance mining).

Strategy (8 NeuronCores, instance dim N=100000 sharded 12500 rows/core):

  Device (per core, fp16 matmuls / fp32 accumulation):
    h1^T = relu(W1^T @ h^T + b1)          [512, 12500]  (D1 on partitions)
    a^T  = tanh(Wa^T @ h1^T + ba)         [256, 12500]
    g^T  = sigmoid(Wb^T @ h1^T + bb)      [256, 12500]
    A    = Wc^T @ (a*g)^T + bc            [1, 12500]   -> output A_raw shard
    E    = exp(A)                          broadcast to 128 partitions via PE
    M_raw[d] = sum_r E_r * h1^T[d, r]      (fused DVE tensor_tensor_reduce)

  Host (numpy fp32):
    - shard + cast h to fp16, pre-transpose per core (so no on-device transpose)
    - A_raw = concat of shards; Z = sum exp(A_raw)
    - pooled M = (sum_c M_raw_c)/Z; logits/Y_prob/Y_hat from M @ Wbag
    - top-k: candidates = global top-64/bottom-64 of device A_raw, then the
      candidate rows are recomputed exactly in fp32 (tiny: 128 rows) and the
      final top-8/bottom-8 + inst_logits come from that exact recompute.
      Device noise (~1e-4) is far below the candidate margin, and the final
      selection/ordering matches the fp32 reference exactly.
"""

import numpy as np
from contextlib import ExitStack

# Problem constants (hardcoded per harness contract).
N, L, D1, D2, TOPK, NCLS = 100000, 1024, 512, 256, 8, 2
NCORES = 8
R = N // NCORES           # 12500 rows per core
BLK = 512                 # rows per block (matmul moving dim / PSUM bank)
NKC = L // 128            # 8 contraction chunks for h @ W1
ND1 = D1 // 128           # 4 D1 chunks
ND2 = D2 // 128           # 2 D2 chunks
NCAND = 64                # top/bottom candidates refined on host

_prog_cache = {}


# ---------------------------------------------------------------------------
# Wait-splitting post-pass: the walrus build in this container rejects
# instructions whose sync_info carries more than one wait ("Too many sync
# wait commands"). Tile freely emits multi-waits; rewrite every instruction
# with k>1 waits into k-1 preceding single-wait NOPs on the same engine.
# Per-engine program order makes this semantically identical.
# ---------------------------------------------------------------------------
def _split_multi_waits(nc):
    import bass_rust
    import concourse.mybir as mybir

    engine_attr = {
        mybir.EngineType.PE: "tensor",
        mybir.EngineType.DVE: "vector",
        mybir.EngineType.Activation: "scalar",
        mybir.EngineType.Pool: "gpsimd",
        mybir.EngineType.SP: "sync",
    }

    def make_wait_nop(engine, wait):
        eng = getattr(nc, engine_attr[engine])
        inst = eng.nop(nofuse=True).ins
        for fn in nc.m.functions:
            for bb in fn.blocks:
                if inst in bb.instructions:
                    bb.instructions.remove(inst)
        inst.sync_info = bass_rust.SyncInfo(on_wait=[wait], on_update=[])
        return inst

    for fn in nc.m.functions:
        for bb in fn.blocks:
            new_insts = []
            for inst in bb.instructions:
                si = inst.sync_info
                if si is not None and si.on_wait and len(si.on_wait) > 1:
                    waits = list(si.on_wait)
                    for w in waits[:-1]:
                        new_insts.append(make_wait_nop(inst.engine, w))
                    inst.sync_info = bass_rust.SyncInfo(
                        on_wait=[waits[-1]], on_update=list(si.on_update or [])
                    )
                new_insts.append(inst)
            bb.instructions[:] = new_insts


def _build_program():
    """Build the per-core SPMD Bass program (same NEFF for all 8 cores)."""
    import concourse.bass as bass
    import concourse.tile as tile
    import concourse.mybir as mybir

    f16, f32 = mybir.dt.float16, mybir.dt.float32
    mult, add = mybir.AluOpType.mult, mybir.AluOpType.add
    AF = mybir.ActivationFunctionType

    nc = bass.Bass("TRN2", debug=False)

    hT = nc.dram_tensor("hT", [L, R], f16, kind="ExternalInput").ap()
    # weights arrive pre-arranged as their SBUF images (one contiguous,
    # descriptor-efficient DMA each): W1f[p, kc*D1+j] = W1[kc*128+p, j] etc.
    W1f = nc.dram_tensor("W1f", [128, NKC * D1], f16, kind="ExternalInput").ap()
    Waf = nc.dram_tensor("Waf", [128, ND1 * D2], f16, kind="ExternalInput").ap()
    Wbf = nc.dram_tensor("Wbf", [128, ND1 * D2], f16, kind="ExternalInput").ap()
    Wcf = nc.dram_tensor("Wcf", [128, ND2], f16, kind="ExternalInput").ap()
    b1f = nc.dram_tensor("b1f", [128, ND1], f32, kind="ExternalInput").ap()
    baf = nc.dram_tensor("baf", [128, ND2], f32, kind="ExternalInput").ap()
    bbf = nc.dram_tensor("bbf", [128, ND2], f32, kind="ExternalInput").ap()
    bcf = nc.dram_tensor("bcf", [1, 1], f32, kind="ExternalInput").ap()

    A_out = nc.dram_tensor("A_out", [1, R], f32, kind="ExternalOutput").ap()
    M_out = nc.dram_tensor("M_out", [128, ND1], f32, kind="ExternalOutput").ap()

    # Block schedule: 512-row blocks; h^T DMAs grouped in quads of blocks
    # (4KB partition lines for descriptor efficiency), prefetched one group
    # ahead of compute.
    blocks = []                                          # (r0, B)
    r = 0
    while r < R:
        B = min(BLK, R - r)
        blocks.append((r, B))
        r += B
    nblk = len(blocks)
    # First two groups are single blocks and the third is a pair, so the
    # PE's first dependencies arrive quickly; steady state uses quads.
    groups = []                                          # group -> block idxs
    b = 0
    while b < nblk:
        want = 1 if len(groups) < 2 else (2 if len(groups) == 2 else 4)
        grp = [bi for bi in range(b, min(b + want, nblk))
               if blocks[bi][1] == BLK or bi == b]
        groups.append(grp)
        b = grp[-1] + 1
    ngrp = len(groups)
    gidx = {}
    goff = {}
    for g, bs in enumerate(groups):
        for j, bi in enumerate(bs):
            gidx[bi] = g
            goff[bi] = blocks[bi][0] - blocks[bs[0]][0]

    with tile.TileContext(nc) as tc, ExitStack() as ctx:
        wpool = ctx.enter_context(tc.tile_pool(name="weights", bufs=1))
        hpool = ctx.enter_context(tc.tile_pool(name="ht", bufs=1))
        cpool = ctx.enter_context(tc.tile_pool(name="compute", bufs=1))
        ppool = ctx.enter_context(tc.tile_pool(name="psum", bufs=1, space="PSUM"))

        # --- persistent weights/biases in SBUF ---
        # W1 tiles are interleaved with the first h^T block so the PE's first
        # matmul dependencies finish loading as early as possible; everything
        # only needed from pipeline stage 2 onward loads afterwards.
        ht_groups = {}   # g -> list of NKC tiles [128, 2*BLK]

        def load_group(g):
            bs = groups[g]
            r0 = blocks[bs[0]][0]
            w = blocks[bs[-1]][0] + blocks[bs[-1]][1] - r0
            tiles = []
            for kc in range(NKC):
                t = hpool.tile([128, 4 * BLK], f16, tag=f"ht{kc}", bufs=2)
                nc.sync.dma_start(
                    t[:, :w], hT[kc * 128:(kc + 1) * 128, r0:r0 + w])
                tiles.append(t)
            ht_groups[g] = tiles

        # W1 (one contiguous 8KB-per-line DMA) then the first two h^T groups.
        W1sb = wpool.tile([128, NKC * D1], f16)          # [k, kc*512 + j]
        nc.sync.dma_start(W1sb[:], W1f[:, :])
        load_group(0)
        Wasb = wpool.tile([128, ND1 * D2], f16)          # [k, d1c*256 + j]
        nc.sync.dma_start(Wasb[:], Waf[:, :])
        Wbsb = wpool.tile([128, ND1 * D2], f16)
        nc.sync.dma_start(Wbsb[:], Wbf[:, :])
        Wcsb = wpool.tile([128, ND2], f16)               # [k, d2c]
        nc.sync.dma_start(Wcsb[:], Wcf[:, :])
        b1sb = wpool.tile([128, ND1], f32)
        nc.sync.dma_start(b1sb[:], b1f[:, :])
        basb = wpool.tile([128, ND2], f32)
        nc.sync.dma_start(basb[:], baf[:, :])
        bbsb = wpool.tile([128, ND2], f32)
        nc.sync.dma_start(bbsb[:], bbf[:, :])
        bcsb = wpool.tile([1, 1], f32)
        nc.sync.dma_start(bcsb[:], bcf[:, :])

        # --- persistent accumulators / staged outputs ---
        A_sb = wpool.tile([1, R], f32)
        Macc = wpool.tile([128, ND1], f32)
        # per-block pooled partial sums; reduced into Macc at the end
        Msum = [wpool.tile([128, nblk], f32, name=f"Msum{d1c}")
                for d1c in range(ND1)]

        # 4-stage software pipeline, one block of skew between stages, so the
        # PE instruction stream never waits on ACT/DVE results of the same
        # block: stage1(b)=load+h1, stage2(b-1)=a/g/s, stage3(b-2)=A+exp,
        # stage4(b-3)=E broadcast + pooled partial.
        st_h1 = {}   # b -> list of 4 h1^T tiles
        st_s = {}    # b -> list of 2 s tiles
        st_E = {}    # b -> Et tile

        def blk_of(b):
            return blocks[b][1]

        for it in range(nblk + 3):
            if it < nblk:
                b, B = it, blk_of(it)
                g, off = gidx[b], goff[b]
                if b == groups[g][0] and g + 1 < ngrp and g + 1 not in ht_groups:
                    load_group(g + 1)
                ht = ht_groups[g]
                h1 = []
                for d1c in range(ND1):
                    ps = ppool.tile([128, BLK], f32, tag="ph1", bufs=3)
                    for kc in range(NKC):
                        lo = kc * D1 + d1c * 128
                        nc.tensor.matmul(ps[:, :B], W1sb[:, lo:lo + 128],
                                         ht[kc][:, off:off + B],
                                         start=(kc == 0), stop=(kc == NKC - 1))
                    t = cpool.tile([128, BLK], f16, tag=f"h1_{d1c}", bufs=5)
                    nc.scalar.activation(t[:, :B], ps[:, :B], AF.Relu,
                                         bias=b1sb[:, d1c:d1c + 1])
                    h1.append(t)
                st_h1[b] = h1

            if 0 <= it - 1 < nblk:
                b, B = it - 1, blk_of(it - 1)
                h1 = st_h1[b]
                s = []
                for d2c in range(ND2):
                    pa = ppool.tile([128, BLK], f32, tag="pag", bufs=3)
                    for d1c in range(ND1):
                        lo = d1c * D2 + d2c * 128
                        nc.tensor.matmul(pa[:, :B], Wasb[:, lo:lo + 128],
                                         h1[d1c][:, :B],
                                         start=(d1c == 0), stop=(d1c == ND1 - 1))
                    at = cpool.tile([128, BLK], f16, tag=f"a_{d2c}", bufs=2)
                    nc.scalar.activation(at[:, :B], pa[:, :B], AF.Tanh,
                                         bias=basb[:, d2c:d2c + 1])

                    pg = ppool.tile([128, BLK], f32, tag="pag", bufs=3)
                    for d1c in range(ND1):
                        lo = d1c * D2 + d2c * 128
                        nc.tensor.matmul(pg[:, :B], Wbsb[:, lo:lo + 128],
                                         h1[d1c][:, :B],
                                         start=(d1c == 0), stop=(d1c == ND1 - 1))
                    gt = cpool.tile([128, BLK], f16, tag=f"g_{d2c}", bufs=2)
                    nc.scalar.activation(gt[:, :B], pg[:, :B], AF.Sigmoid,
                                         bias=bbsb[:, d2c:d2c + 1])

                    st = cpool.tile([128, BLK], f16, tag=f"s_{d2c}", bufs=3)
                    nc.vector.tensor_mul(st[:, :B], at[:, :B], gt[:, :B])
                    s.append(st)
                st_s[b] = s

            if 0 <= it - 2 < nblk:
                b = it - 2
                r0, B = blocks[b]
                s = st_s.pop(b)
                pA = ppool.tile([1, BLK], f32, tag="pA", bufs=2)
                for d2c in range(ND2):
                    nc.tensor.matmul(pA[:1, :B], Wcsb[:, d2c:d2c + 1],
                                     s[d2c][:, :B],
                                     start=(d2c == 0), stop=(d2c == ND2 - 1))
                nc.scalar.activation(A_sb[:1, r0:r0 + B], pA[:1, :B], AF.Identity,
                                     bias=bcsb[:1, 0:1])
                Et = cpool.tile([1, BLK], f16, tag="E", bufs=3)
                nc.scalar.activation(Et[:1, :B], pA[:1, :B], AF.Exp,
                                     bias=bcsb[:1, 0:1])
                st_E[b] = Et

            if 0 <= it - 3 < nblk:
                b, B = it - 3, blk_of(it - 3)
                h1 = st_h1.pop(b)
                Et = st_E.pop(b)
                # broadcast E to all 128 partitions via SWDGE DMA (0-step
                # free dim on the source) — keeps the PE stream pure matmul
                Es = cpool.tile([128, BLK], f16, tag="Es", bufs=2)
                src = (Et[0:1, :B].rearrange("p (a f) -> p a f", a=1)
                       .broadcast_to((1, 128, B)))
                nc.sync.dma_start(Es[:, :B], src)
                for d1c in range(ND1):
                    tmp = cpool.tile([128, BLK], f32, tag="pooltmp", bufs=2)
                    nc.vector.scalar_tensor_tensor(
                        out=tmp[:, :B], in0=h1[d1c][:, :B], scalar=1.0,
                        in1=Es[:, :B], op0=mult, op1=mult,
                        accum_out=Msum[d1c][:, b:b + 1])

        for d1c in range(ND1):
            nc.vector.reduce_sum(Macc[:, d1c:d1c + 1], Msum[d1c][:, :],
                                 axis=mybir.AxisListType.X)

        nc.sync.dma_start(A_out[:, :], A_sb[:1, :])
        nc.sync.dma_start(M_out[:, :], Macc[:, :])

    _split_multi_waits(nc)
    return nc


def _run_device(h):
    """Shard/cast/transpose h, run the SPMD kernel, return (A_raw[N], M_raw[512])
    plus the per-core weight inputs captured in _run_device.weights."""
    from concourse.bass_utils import run_bass_kernel_spmd

    if "nc" not in _prog_cache:
        _prog_cache["nc"] = _build_program()
    nc = _prog_cache["nc"]

    w = _run_device.weights
    in_maps = []
    for c in range(NCORES):
        shard = h[c * R:(c + 1) * R, :]
        hT_c = np.ascontiguousarray(shard.astype(np.float16).T)
        in_maps.append({"hT": hT_c, **w})

    try:
        res = run_bass_kernel_spmd(nc, in_maps, core_ids=list(range(NCORES)))
    except Exception:
        # The axon-tunneled runtime occasionally reports the accelerator as
        # unrecoverable right after a profiled session; a retry on a fresh
        # dispatch clears it.
        import time
        time.sleep(2.0)
        res = run_bass_kernel_spmd(nc, in_maps, core_ids=list(range(NCORES)))
    A_raw = np.concatenate([res.results[c]["A_out"][0] for c in range(NCORES)])
    M_raw = np.zeros(D1, np.float64)
    for c in range(NCORES):
        M_raw += res.results[c]["M_out"].T.reshape(D1)
    return A_raw, M_raw


_run_device.weights = None


def kernel(h, W1, b1, Wa, ba, Wb, bb, Wc, bc, Wbag, bbag, Winst, binst, label):
    h = np.asarray(h, dtype=np.float32)
    W1 = np.asarray(W1, dtype=np.float32)
    b1 = np.asarray(b1, dtype=np.float32)
    Wa = np.asarray(Wa, dtype=np.float32)
    ba = np.asarray(ba, dtype=np.float32)
    Wb = np.asarray(Wb, dtype=np.float32)
    bb = np.asarray(bb, dtype=np.float32)
    Wc = np.asarray(Wc, dtype=np.float32)
    bc = np.asarray(bc, dtype=np.float32)
    Wbag = np.asarray(Wbag, dtype=np.float32)
    bbag = np.asarray(bbag, dtype=np.float32)
    Winst = np.asarray(Winst, dtype=np.float32)
    binst = np.asarray(binst, dtype=np.float32)

    def sbuf_image(W, nchunk):
        # [nchunk*128, F] -> [128, nchunk*F]: img[p, c*F+j] = W[c*128+p, j]
        F = W.shape[1]
        return np.ascontiguousarray(
            W.reshape(nchunk, 128, F).transpose(1, 0, 2).reshape(128, nchunk * F)
        ).astype(np.float16)

    _run_device.weights = {
        "W1f": sbuf_image(W1, NKC),
        "Waf": sbuf_image(Wa, ND1),
        "Wbf": sbuf_image(Wb, ND1),
        "Wcf": sbuf_image(Wc, ND2),
        "b1f": np.ascontiguousarray(b1.reshape(ND1, 128).T.astype(np.float32)),
        "baf": np.ascontiguousarray(ba.reshape(ND2, 128).T.astype(np.float32)),
        "bbf": np.ascontiguousarray(bb.reshape(ND2, 128).T.astype(np.float32)),
        "bcf": bc.reshape(1, 1).astype(np.float32),
    }

    A_raw_dev, M_raw = _run_device(h)  # [N] f32 (device), [512] f64 partials

    # --- bag branch (host fp32, negligible cost) ---
    Aexp = np.exp(A_raw_dev.astype(np.float64))
    Z = Aexp.sum()
    M = (M_raw / Z).astype(np.float32)                    # [512] pooled vector
    logits = (M @ Wbag + bbag).reshape(1, NCLS).astype(np.float32)
    lmax = logits.max(axis=1, keepdims=True)
    e = np.exp(logits - lmax)
    Y_prob = (e / e.sum(axis=1, keepdims=True)).astype(np.float32)
    Y_hat = np.argmax(logits, axis=1).astype(np.int32)

    # --- instance branch: exact fp32 refinement of candidates ---
    top_cand = np.argpartition(-A_raw_dev, NCAND - 1)[:NCAND]
    bot_cand = np.argpartition(A_raw_dev, NCAND - 1)[:NCAND]
    cand = np.unique(np.concatenate([top_cand, bot_cand]))

    h_sel = h[cand]                                        # [|cand|, 1024]
    h1_sel = np.maximum(h_sel @ W1 + b1, 0.0).astype(np.float32)
    a_sel = np.tanh(h1_sel @ Wa + ba)
    g_sel = 1.0 / (1.0 + np.exp(-(h1_sel @ Wb + bb)))
    A_sel = ((a_sel * g_sel).astype(np.float32) @ Wc + bc).reshape(-1).astype(np.float32)

    # top_k on softmax scores == top_k on A (softmax monotonic); lax.top_k
    # breaks ties toward the lower index.
    order_desc = np.lexsort((cand, -A_sel))
    order_asc = np.lexsort((cand, A_sel))
    top_rows = order_desc[:TOPK]
    bot_rows = order_asc[:TOPK]
    sel_rows = np.concatenate([top_rows, bot_rows])
    all_inst = h1_sel[sel_rows]                            # [2K, 512] exact fp32
    inst_logits = (all_inst @ Winst + binst).astype(np.float32)

    A_raw = A_raw_dev.reshape(1, N).astype(np.float32)
    return (logits, Y_prob, Y_hat, A_raw, inst_logits)
